# revision 3
# baseline (speedup 1.0000x reference)
"""Trainium2 Bass kernel v2 for nn_BasicTransformerBlock (sparse attention).

Instruction-count-minimized redesign. This environment prices execution per
instruction (~60-90us per ldweights+matmul pair, ~40-50us DVE/ACT, ~20-40us
DMA), so the kernel minimizes weighted instruction count:

  - fp8(e4m3) DoubleRow matmuls (256-wide contract) for every dense GEMM:
    weights stored as fp8(16*W), activations fp8 at natural scale, epilogues
    descale by 1/16 (one fp8 factor) or 1/256 (two). Contract-640 GEMMs are
    zero-padded to 6 k-tiles = 3 DR matmuls.
  - Sparse-causal attention (B): head-PAIR packed score matmuls (block-diag
    zero-padded Q in the DoubleRow slot dim computes two heads' scores per
    instruction), exp->fp8 P, DoubleRow PV with the softmax denominator as
    an 81st V row.
  - Per-head outputs divided by the denominator and packed into [640, tok]
    tiles (DVE partition-base shifts), so output projections run as dense
    640-contract DR GEMMs with row-permuted weights.
  - Temporal attention (D): head-pair packed scores; block-diag seq mask
    multiplied in while scattering into slot-paired zero-padded fp8 P for
    DoubleRow PV (two 128-token tiles per instruction).
  - LN stats via DVE presum over k-tiles + ones-matmuls; frame->seq reshard
    via bf16 AllToAll.
"""

import math

import numpy as np
import ml_dtypes

import concourse.bass as bass
import concourse.bacc as bacc
import concourse.tile as tile
from concourse import mybir
from concourse.bass_utils import run_bass_kernel_spmd

BF16 = mybir.dt.bfloat16
F32 = mybir.dt.float32
FP8 = mybir.dt.float8e4
AF = mybir.ActivationFunctionType
OP = mybir.AluOpType
DR = mybir.MatmulPerfMode.DoubleRow
E4 = ml_dtypes.float8_e4m3

H, DH, C, F, D = 8, 80, 640, 16, 1024
INNER = 4 * C            # 2560
NI = INNER // 128        # 20
NCORES = 8
KT = C // 128            # 5
NQ = 2 * D               # 2048 own tokens / core
NKV = 3 * D              # 3072 kv-context tokens / core
NU = 4 * D               # 4096 union tokens / core
EPS = 1e-5
RG = [list(range(NCORES))]

_BUILD_CACHE = {}


def _head_rows(h):
    """Head h's 80 packed rows as (m-tile, row0, head-off, len) chunks."""
    out = []
    c0 = h * DH
    off = 0
    while off < DH:
        m = (c0 + off) // 128
        r = (c0 + off) % 128
        ln = min(DH - off, 128 - r)
        out.append((m, r, off, ln))
        off += ln
    return out


def _ap(view, ap, doff=0):
    return bass.AP(tensor=view.tensor, offset=view.offset + doff, ap=ap)


def _build(reps=1, use_cc=True):
    key = (reps, use_cc)
    if key in _BUILD_CACHE:
        return _BUILD_CACHE[key]
    nc = bacc.Bacc("TRN2", target_bir_lowering=False, debug=False,
                   num_devices=NCORES)

    def din(name, shape, dt):
        return nc.dram_tensor(name, shape, dt, kind="ExternalInput").ap()

    xub_d = din("xub", [KT, 128, NU], BF16)
    wq_d = din("wq", [128, 6, C], FP8)       # fp8(16*w), kt-padded to 6
    wk_d = din("wk", [128, 6, C], FP8)
    wv_d = din("wv", [128, 6, C], FP8)
    wo_d = din("wo", [128, 6, C], FP8)       # packed (h,dh) row order
    wf1_d = din("wf1", [128, 6, 2 * INNER], FP8)
    wf2_d = din("wf2", [128, NI, C], FP8)
    wqt_d = din("wqt", [128, 6, C], FP8)
    wkt_d = din("wkt", [128, 6, C], FP8)
    wvt_d = din("wvt", [128, 6, C], FP8)
    wot_d = din("wot", [128, 6, C], FP8)
    bq_d = din("bq", [C], F32)
    bk_d = din("bk", [C], F32)
    bv16_d = din("bv16", [C], F32)           # 16*bv
    bf1g_d = din("bf1g", [INNER], F32)
    bf1a16_d = din("bf1a16", [INNER], F32)   # 16*bf1[:INNER]
    bqt_d = din("bqt", [C], F32)
    bkt_d = din("bkt", [C], F32)
    bvt16_d = din("bvt16", [C], F32)
    mask_d = din("mask", [128, 128], BF16)   # eye(8) x ones(16,16)
    out_d = nc.dram_tensor("out", [KT, 128, NQ], F32,
                           kind="ExternalOutput").ap()

    with tile.TileContext(nc) as tc:
        with tc.tile_pool(name="const", bufs=1) as constp, \
             tc.tile_pool(name="dram", bufs=1, space="DRAM") as dramp:

            ones_b = constp.tile([128, 128], BF16)
            nc.vector.memset(ones_b, 1.0)
            epst = constp.tile([128, 1], F32)
            nc.vector.memset(epst, EPS)
            i256 = constp.tile([128, 1], F32, tag="i256", name="i256")
            nc.vector.memset(i256, 1.0 / 256)
            mask_t = constp.tile([128, 128], BF16, tag="mask", name="mask")
            nc.sync.dma_start(out=mask_t, in_=mask_d)

            def bias_tile(b_d, ncols):
                t = constp.tile([128, ncols], F32, tag=f"b_{b_d.tensor.name}",
                                name=f"b_{b_d.tensor.name}")
                nc.sync.dma_start(out=t, in_=b_d.rearrange("(m p) -> p m",
                                                           p=128))
                return t

            bq_t = bias_tile(bq_d, KT)
            bk_t = bias_tile(bk_d, KT)
            bv16_t = bias_tile(bv16_d, KT)
            bf1g_t = bias_tile(bf1g_d, NI)
            bf1a16_t = bias_tile(bf1a16_d, NI)
            bqt_t = bias_tile(bqt_d, KT)
            bkt_t = bias_tile(bkt_d, KT)
            bvt16_t = bias_tile(bvt16_d, KT)

            def ln_stats(pool, psp, wkp, xs, ntok, tag):
                """xs: KT SBUF views [128, ntok] bf16 -> (M, R) bf16."""
                nbl = ntok // 2048
                M = pool.tile([128, ntok], BF16, tag=f"M_{tag}",
                              name=f"M_{tag}")
                S = wkp.tile([128, ntok], F32, tag=f"S_{tag}",
                             name=f"S_{tag}")
                for bl in range(nbl):
                    bsl = slice(bl * 2048, (bl + 1) * 2048)
                    ps_s = psp.tile([128, 2048], F32, tag="pss")
                    ps_q = psp.tile([128, 2048], F32, tag="psq")
                    xsum = wkp.tile([128, 2048], BF16, tag="ln_xsum",
                                    name=f"xsum_{tag}{bl}", bufs=1)
                    nc.vector.tensor_add(xsum, xs[0][:, bsl], xs[1][:, bsl])
                    for kt in range(2, KT):
                        nc.vector.tensor_add(xsum, xsum, xs[kt][:, bsl])
                    sqs = wkp.tile([128, 2048], BF16, tag="ln_sqs",
                                   name=f"sqs_{tag}{bl}", bufs=1)
                    nc.vector.tensor_mul(sqs, xs[0][:, bsl], xs[0][:, bsl])
                    tmp = wkp.tile([128, 2048], BF16, tag="ln_tmp",
                                   name=f"lntmp_{tag}{bl}", bufs=1)
                    for kt in range(1, KT):
                        nc.vector.tensor_mul(tmp, xs[kt][:, bsl],
                                             xs[kt][:, bsl])
                        nc.vector.tensor_add(sqs, sqs, tmp)
                    for ch in range(4):
                        sl = slice(ch * 512, (ch + 1) * 512)
                        nc.tensor.matmul(ps_s[:, sl], ones_b, xsum[:, sl],
                                         start=True, stop=True)
                        nc.tensor.matmul(ps_q[:, sl], ones_b, sqs[:, sl],
                                         start=True, stop=True)
                    nc.scalar.activation(out=M[:, bsl], in_=ps_s,
                                         func=AF.Identity, scale=1.0 / C)
                    nc.scalar.activation(out=S[:, bsl], in_=ps_q,
                                         func=AF.Identity, scale=1.0 / C)
                    msq = wkp.tile([128, 2048], BF16, tag="ln_tmp",
                                   name=f"msq_{tag}{bl}", bufs=1)
                    nc.vector.tensor_mul(msq, M[:, bsl], M[:, bsl])
                    nc.vector.tensor_sub(S[:, bsl], S[:, bsl], msq)
                nc.scalar.activation(out=S, in_=S, func=AF.Sqrt, bias=epst)
                R = pool.tile([128, ntok], BF16, tag=f"R_{tag}",
                              name=f"R_{tag}")
                with nc.allow_low_precision(reason="rstd bf16"):
                    nc.vector.reciprocal(out=R, in_=S)
                return M, R

            def make_x8(pool, wkp, psp, xs, ntok, tag):
                """LN + fp8 cast -> x8 [128, 6, ntok] fp8 (tile 5 zeroed)."""
                M, R = ln_stats(pool, psp, wkp, xs, ntok, tag)
                x8 = pool.tile([128, 6, ntok], FP8, tag=f"x8_{tag}",
                               name=f"x8_{tag}")
                nc.vector.memset(x8[:, 5, :], 0.0)
                for kt in range(KT):
                    tmp = wkp.tile([128, ntok], BF16, tag="x8tmp",
                                   name=f"x8tmp_{tag}{kt}", bufs=1)
                    nc.vector.tensor_sub(tmp, xs[kt], M)
                    nc.vector.tensor_mul(x8[:, kt, :], tmp, R)
                return x8

            def proj_dr(psp, w_t, x8, ncols, xoff=0):
                """Yield (m, [(ch0, width, psum)]) of a 640-contract DR GEMM.
                w_t [128, 6, *] fp8 ; x8 [128, 6, ntok] fp8."""
                for m in range(KT):
                    msl = slice(m * 128, (m + 1) * 128)
                    res = []
                    for ch0 in range(0, ncols, 2048):
                        w = min(2048, ncols - ch0)
                        pq = psp.tile([128, 2048], F32, tag="pq", bufs=2)
                        for qc in range(w // 512):
                            osl = slice(qc * 512, (qc + 1) * 512)
                            xsl = slice(xoff + ch0 + qc * 512,
                                        xoff + ch0 + (qc + 1) * 512)
                            for g in range(3):
                                nc.tensor.matmul(
                                    pq[:, osl], w_t[:, 2 * g:2 * g + 2, msl],
                                    x8[:, 2 * g:2 * g + 2, xsl],
                                    start=(g == 0), stop=(g == 2),
                                    perf_mode=DR)
                        res.append((ch0, w, pq))
                    yield m, res

            def load_w(pool, w_d, name, ncols=C):
                t = pool.tile([128, 6, ncols], FP8, tag=f"w_{name}",
                              name=f"w_{name}")
                nc.sync.dma_start(out=t, in_=w_d)
                return t

            def vload(vt, src_dr, ntiles, toff=0):
                """Per-head transposed V loads from token-major [NT, C] DRAM
                into [128, ntiles, 648] (81-row layout, row 80 = ones)."""
                for h in range(H):
                    nc.sync.dma_start(
                        out=vt[:, :, h * 84:h * 84 + 80],
                        in_=_ap(src_dr[:, :], [[C, 128], [128 * C, ntiles],
                                               [1, 80]],
                                doff=toff * C + h * DH))
                nc.vector.memset(
                    vt.rearrange("p w (h d) -> p (w h) d", d=84)[:, :, 80],
                    1.0)

            def emit(it):
                qh_dr = dramp.tile([C, NQ], FP8, tag="qh_dr", name="qh_dr")
                kh_dr = dramp.tile([C, NKV], FP8, tag="kh_dr", name="kh_dr")
                vd_dr = dramp.tile([NKV, C], FP8, tag="vd_dr", name="vd_dr")
                qt_dr = dramp.tile([C, NQ], FP8, tag="qt_dr", name="qt_dr")
                kt_dr = dramp.tile([C, NQ], FP8, tag="kt_dr", name="kt_dr")
                vt_dr = dramp.tile([NQ, C], FP8, tag="vt_dr", name="vt_dr")
                a2a_in = dramp.tile([NCORES, C, 256], BF16, tag="a2a_in",
                                    name="a2a_in")
                a2a_out = dramp.tile([NCORES, C, 256], BF16, tag="a2a_out",
                                     name="a2a_out")

                with tc.tile_pool(name="pH", bufs=1) as pH:
                    h1 = pH.tile([128, KT, NQ], BF16, tag="h1")
                    with tc.tile_pool(name="pAB", bufs=1) as pAB:
                        xu = pAB.tile([128, KT, NU], BF16, tag="xu")
                        nc.sync.dma_start(
                            out=xu, in_=xub_d.rearrange("k p c -> p k c"))

                        # ===== Phase A: LN1 + q/k/v projections =====
                        with tc.tile_pool(name="pA", bufs=1) as pA, \
                             tc.tile_pool(name="pAw", bufs=1) as pAw:
                            with tc.tile_pool(name="psS", bufs=1,
                                              space="PSUM") as psS:
                                xus = [xu[:, kt, :] for kt in range(KT)]
                                x8 = make_x8(pA, pAw, psS, xus, NU, "ln1")
                            wq_t = load_w(pA, wq_d, "wq")
                            wk_t = load_w(pA, wk_d, "wk")
                            wv_t = load_w(pA, wv_d, "wv")
                            with tc.tile_pool(name="psP", bufs=1,
                                              space="PSUM") as psP:
                                for m, res in proj_dr(psP, wq_t, x8, NQ,
                                                      xoff=NU - NQ):
                                    (_, w, pq), = res
                                    qe = pAw.tile([128, NQ], FP8, tag="qe",
                                                  bufs=2)
                                    nc.scalar.activation(
                                        out=qe, in_=pq, func=AF.Identity,
                                        scale=1.0 / 16,
                                        bias=bq_t[:, m:m + 1])
                                    nc.sync.dma_start(
                                        out=qh_dr[m * 128:(m + 1) * 128, :],
                                        in_=qe)
                                for m, res in proj_dr(psP, wk_t, x8, NKV):
                                    for ch0, w, pk in res:
                                        ke = pAw.tile([128, 2048], FP8,
                                                      tag="ke", bufs=1)
                                        nc.scalar.activation(
                                            out=ke[:, :w], in_=pk[:, :w],
                                            func=AF.Identity,
                                            scale=1.0 / 16,
                                            bias=bk_t[:, m:m + 1])
                                        nc.sync.dma_start(
                                            out=kh_dr[m * 128:(m + 1) * 128,
                                                      ch0:ch0 + w],
                                            in_=ke[:, :w])
                                for m, res in proj_dr(psP, wv_t, x8, NKV):
                                    for ch0, w, pv in res:
                                        vee = pAw.tile([128, 2048], FP8,
                                                       tag="ve", bufs=1)
                                        nc.scalar.activation(
                                            out=vee[:, :w], in_=pv[:, :w],
                                            func=AF.Identity,
                                            bias=bv16_t[:, m:m + 1])
                                        nc.sync.dma_start(
                                            out=_ap(vd_dr[:, :],
                                                    [[1, 128], [C, w]],
                                                    doff=(ch0 * C
                                                          + m * 128)),
                                            in_=vee[:, :w])

                        # ===== Phase B: sparse-causal attention =====
                        with tc.tile_pool(name="pB", bufs=1) as pB, \
                             tc.tile_pool(name="pBw", bufs=1) as pBw:
                            wo_t = load_w(pB, wo_d, "wo")
                            # V [keys-part] tiles: frame0 (8) + per-fi prev
                            v80 = pB.tile([128, 8, H * 84], FP8,
                                          tag="v80", name="v80")
                            vload(v80, vd_dr, 8)
                            v8p = []
                            for fi in range(2):
                                t = pB.tile([128, 8, H * 84], FP8,
                                            tag=f"v8p{fi}", name=f"v8p{fi}")
                                vload(t, vd_dr, 8, toff=(1 + fi) * D)
                                v8p.append(t)

                            # per-head K/Q tiles [80, h, tok] fp8
                            KH0 = pB.tile([80, 8, D], FP8, tag="KH0",
                                          name="KH0")
                            nc.sync.dma_start(
                                out=KH0,
                                in_=_ap(kh_dr[:, :],
                                        [[NKV, 80], [DH * NKV, 8],
                                         [1, D]]))
                            KHp = [pB.tile([80, 8, D], FP8, tag=f"KHp{fi}",
                                           name=f"KHp{fi}")
                                   for fi in range(2)]
                            QS = [pB.tile([80, 8, D], FP8, tag=f"QS{fi}",
                                          name=f"QS{fi}")
                                  for fi in range(2)]
                            for fi in range(2):
                                nc.sync.dma_start(
                                    out=KHp[fi],
                                    in_=_ap(kh_dr[:, :],
                                            [[NKV, 80], [DH * NKV, 8],
                                             [1, D]],
                                            doff=(1 + fi) * D))
                                nc.sync.dma_start(
                                    out=QS[fi],
                                    in_=_ap(qh_dr[:, :],
                                            [[NQ, 80], [DH * NQ, 8],
                                             [1, D]],
                                            doff=fi * D))
                            attD8 = pB.tile([128, 6, NQ], FP8, tag="attD8",
                                            name="attD8")
                            nc.vector.memset(attD8[:, 5, :], 0.0)
                            with tc.tile_pool(name="psB", bufs=1,
                                              space="PSUM") as psB:
                                for h in range(H):
                                    h84 = h * 84
                                    pvp = psB.tile([81, 2048], F32,
                                                   tag="pvp", name="pvp")
                                    for fi in range(2):
                                        for qc in range(2):
                                            csl = slice(
                                                fi * D + qc * 512,
                                                fi * D + (qc + 1) * 512)
                                            qsl = slice(qc * 512,
                                                        (qc + 1) * 512)
                                            for wg in range(4):
                                                vsrc = (v80 if wg < 2
                                                        else v8p[fi])
                                                kh = (KH0 if wg < 2
                                                      else KHp[fi])
                                                pP = psB.tile(
                                                    [128, 4, 512], F32,
                                                    tag="pP", name="pP")
                                                for t4 in range(4):
                                                    t = (wg % 2) * 4 + t4
                                                    nc.tensor.matmul(
                                                        pP[:, t4, :],
                                                        kh[:, h,
                                                           t * 128:
                                                           (t + 1) * 128],
                                                        QS[fi][:, h, qsl],
                                                        start=True,
                                                        stop=True)
                                                P8 = pBw.tile(
                                                    [128, 4, 512], FP8,
                                                    tag="P8", bufs=1)
                                                nc.scalar.activation(
                                                    out=P8, in_=pP,
                                                    func=AF.Exp)
                                                for half in range(2):
                                                    wt = ((wg % 2) * 4
                                                          + half * 2)
                                                    nc.tensor.matmul(
                                                        pvp[:, csl],
                                                        vsrc[:, wt:wt + 2,
                                                             h84:
                                                             h84 + 81],
                                                        P8[:, 2 * half:
                                                           2 * half + 2,
                                                           :],
                                                        start=(wg == 0 and
                                                               half == 0),
                                                        stop=(wg == 3 and
                                                              half == 1),
                                                        perf_mode=DR)
                                    attP = pBw.tile([81, 2048], BF16,
                                                    tag="attP", bufs=1)
                                    nc.vector.tensor_copy(attP, pvp)
                                    dnm = pBw.tile([1, 2048], BF16,
                                                   tag="dnm", bufs=1)
                                    nc.sync.dma_start(out=dnm,
                                                      in_=attP[80:81, :])
                                    dnr = pBw.tile([1, 2048], F32,
                                                   tag="dnr", bufs=1)
                                    nc.vector.reciprocal(out=dnr, in_=dnm)
                                    aB = pBw.tile([80, 2048], F32,
                                                  tag="aB", bufs=1)
                                    nc.gpsimd.partition_broadcast(
                                        aB, dnr[0:1, :], channels=80)
                                    aD8 = pBw.tile([80, 2048], FP8,
                                                   tag="aD8", bufs=1)
                                    nc.vector.tensor_mul(
                                        aD8, attP[0:80, :], aB)
                                    for (m, r, off, ln) in _head_rows(h):
                                        nc.sync.dma_start(
                                            out=attD8[r:r + ln, m, :],
                                            in_=aD8[off:off + ln, :])
                            with tc.tile_pool(name="psO", bufs=1,
                                              space="PSUM") as psO:
                                for m, res in proj_dr(psO, wo_t, attD8,
                                                      NQ):
                                    (_, w, po), = res
                                    nc.vector.scalar_tensor_tensor(
                                        out=h1[:, m, :], in0=po,
                                        scalar=i256[:, 0:1],
                                        in1=xu[:, m, NU - NQ:],
                                        op0=OP.mult, op1=OP.add)

                    # ===== Phase C: LN3 + GEGLU FF =====
                    with tc.tile_pool(name="pC", bufs=1) as pC, \
                         tc.tile_pool(name="pCw", bufs=1) as pCw:
                        with tc.tile_pool(name="psS3", bufs=1,
                                          space="PSUM") as psS3:
                            h1s = [h1[:, kt, :] for kt in range(KT)]
                            x38 = make_x8(pC, pCw, psS3, h1s, NQ, "ln3")
                        wf1_t = load_w(pC, wf1_d, "wf1", ncols=2 * INNER)
                        ffin8 = pC.tile([128, NI, NQ], FP8, tag="ffin8")
                        with tc.tile_pool(name="psF", bufs=1,
                                          space="PSUM") as psF:
                            for j in range(NI):
                                psa = psF.tile([128, 2048], F32, tag="psa")
                                psg = psF.tile([128, 2048], F32, tag="psg")
                                for qc in range(4):
                                    osl = slice(qc * 512, (qc + 1) * 512)
                                    for g in range(3):
                                        nc.tensor.matmul(
                                            psa[:, osl],
                                            wf1_t[:, 2 * g:2 * g + 2,
                                                  j * 128:(j + 1) * 128],
                                            x38[:, 2 * g:2 * g + 2, osl],
                                            start=(g == 0), stop=(g == 2),
                                            perf_mode=DR)
                                    for g in range(3):
                                        nc.tensor.matmul(
                                            psg[:, osl],
                                            wf1_t[:, 2 * g:2 * g + 2,
                                                  INNER + j * 128:
                                                  INNER + (j + 1) * 128],
                                            x38[:, 2 * g:2 * g + 2, osl],
                                            start=(g == 0), stop=(g == 2),
                                            perf_mode=DR)
                                gg = pCw.tile([128, 2048], BF16, tag="gg",
                                              bufs=1)
                                nc.scalar.activation(
                                    out=gg, in_=psg, func=AF.Gelu,
                                    scale=1.0 / 16,
                                    bias=bf1g_t[:, j:j + 1])
                                nc.vector.scalar_tensor_tensor(
                                    out=ffin8[:, j, :], in0=psa,
                                    scalar=bf1a16_t[:, j:j + 1], in1=gg,
                                    op0=OP.add, op1=OP.mult)
                        wf2_t = pC.tile([128, NI, C], FP8, tag="wf2")
                        nc.sync.dma_start(out=wf2_t, in_=wf2_d)
                        with tc.tile_pool(name="psF2", bufs=1,
                                          space="PSUM") as psF2:
                            for m in range(KT):
                                msl = slice(m * 128, (m + 1) * 128)
                                pf = psF2.tile([128, 2048], F32, tag="pf",
                                               bufs=2)
                                for qc in range(4):
                                    osl = slice(qc * 512, (qc + 1) * 512)
                                    for g in range(10):
                                        nc.tensor.matmul(
                                            pf[:, osl],
                                            wf2_t[:, 2 * g:2 * g + 2, msl],
                                            ffin8[:, 2 * g:2 * g + 2, osl],
                                            start=(g == 0), stop=(g == 9),
                                            perf_mode=DR)
                                nc.vector.scalar_tensor_tensor(
                                    out=h1[:, m, :], in0=pf,
                                    scalar=i256[:, 0:1], in1=h1[:, m, :],
                                    op0=OP.mult, op1=OP.add)

                    # ===== reshard: frames -> spatial slice =====
                    for kt in range(KT):
                        hv = h1[:, kt, :].rearrange(
                            "p (fl j s) -> p j fl s", fl=2, j=NCORES)
                        for j in range(NCORES):
                            nc.sync.dma_start(
                                out=a2a_in[j, kt * 128:(kt + 1) * 128, :],
                                in_=hv[:, j])
                if use_cc:
                    nc.gpsimd.collective_compute(
                        "AllToAll", OP.bypass, replica_groups=RG,
                        ins=[a2a_in[:, :, :]], outs=[a2a_out[:, :, :]])
                else:
                    nc.sync.dma_start(out=a2a_out[:, :, :],
                                      in_=a2a_in[:, :, :])

                # ===== Phase D: temporal attention =====
                with tc.tile_pool(name="pD", bufs=1) as pD, \
                     tc.tile_pool(name="pDw", bufs=1) as pDw:
                    ht = pD.tile([128, KT, NQ], BF16, tag="ht")
                    for kt in range(KT):
                        hfm = pDw.tile([128, NQ], BF16, tag="hfm", bufs=1)
                        for j in range(NCORES):
                            nc.sync.dma_start(
                                out=hfm[:, j * 256:(j + 1) * 256],
                                in_=a2a_out[j, kt * 128:(kt + 1) * 128, :])
                        nc.vector.tensor_copy(
                            ht[:, kt, :].rearrange("p (s f) -> p f s", f=F),
                            hfm.rearrange("p (f s) -> p f s", f=F))
                    with tc.tile_pool(name="psSt", bufs=1,
                                      space="PSUM") as psSt:
                        hts = [ht[:, kt, :] for kt in range(KT)]
                        xt8 = make_x8(pD, pDw, psSt, hts, NQ, "lnt")

                    wqt_t = load_w(pD, wqt_d, "wqt")
                    wkt_t = load_w(pD, wkt_d, "wkt")
                    wvt_t = load_w(pD, wvt_d, "wvt")
                    wot_t = load_w(pD, wot_d, "wot")
                    with tc.tile_pool(name="psD1", bufs=1,
                                      space="PSUM") as psD1:
                        for w_t, b_t, dst, s16 in (
                                (wqt_t, bqt_t, qt_dr, True),
                                (wkt_t, bkt_t, kt_dr, True),
                                (wvt_t, bvt16_t, vt_dr, False)):
                            for m, res in proj_dr(psD1, w_t, xt8, NQ):
                                (_, w, pq), = res
                                qe = pDw.tile([128, NQ], FP8, tag="qte",
                                              bufs=1)
                                if s16:
                                    nc.scalar.activation(
                                        out=qe, in_=pq, func=AF.Identity,
                                        scale=1.0 / 16,
                                        bias=b_t[:, m:m + 1])
                                else:
                                    nc.scalar.activation(
                                        out=qe, in_=pq, func=AF.Identity,
                                        bias=b_t[:, m:m + 1])
                                if s16:
                                    nc.sync.dma_start(
                                        out=dst[m * 128:(m + 1) * 128, :],
                                        in_=qe)
                                else:
                                    nc.sync.dma_start(
                                        out=_ap(dst[:, :],
                                                [[1, 128], [C, NQ]],
                                                doff=m * 128),
                                        in_=qe)

                    vt8 = pD.tile([128, 16, H * 84], FP8, tag="vt8",
                                  name="vt8")
                    vload(vt8, vt_dr, 16)
                    KPt = pD.tile([80, 2, 4, 16 * 128], FP8, tag="KPt",
                                  name="KPt")
                    QPt = pD.tile([80, 2, 4, 16, 256], FP8, tag="QPt",
                                  name="QPt")
                    nc.vector.memset(QPt, 0.0)
                    for hp in range(4):
                        for i in range(2):
                            r0 = (2 * hp + i) * DH
                            nc.sync.dma_start(
                                out=KPt[:, i, hp, :],
                                in_=kt_dr[r0:r0 + 80, :])
                            nc.sync.dma_start(
                                out=QPt[:, i, hp, :,
                                        i * 128:(i + 1) * 128],
                                in_=qt_dr[r0:r0 + 80, :]
                                .rearrange("d (st c) -> d st c", c=128))
                    attDt8 = pD.tile([128, 6, NQ], FP8, tag="attDt8",
                                     name="attDt8")
                    nc.vector.memset(attDt8[:, 5, :], 0.0)
                    # zero-padded slot-paired P, reused across heads
                    Ppad = [pD.tile([128, 8, 2, 256], FP8, tag=f"Ppad{i}",
                                    name=f"Ppad{i}") for i in range(2)]
                    for i in range(2):
                        nc.vector.memset(Ppad[i], 0.0)
                    with tc.tile_pool(name="psD2", bufs=1,
                                      space="PSUM") as psD2:
                        for hp in range(4):
                            E = pDw.tile([128, 16, 256], BF16, tag="E",
                                         bufs=1)
                            for half in range(2):
                                psc = psD2.tile([128, 8, 256], F32,
                                                tag="psc")
                                for st8 in range(8):
                                    st = half * 8 + st8
                                    nc.tensor.matmul(
                                        psc[:, st8, :],
                                        KPt[:, :, hp,
                                            st * 128:(st + 1) * 128],
                                        QPt[:, :, hp, st, :],
                                        start=True, stop=True,
                                        perf_mode=DR)
                                nc.scalar.activation(
                                    out=E[:, half * 8:(half + 1) * 8, :],
                                    in_=psc, func=AF.Exp)
                            for i in range(2):
                                h = 2 * hp + i
                                PT = Ppad[i]
                                # masked scatter into slot-diag of Ppad:
                                # PT[p, pr, j, j*128+c] =
                                #     E[p, 2pr+j, i*128+c] * mask[p, c]
                                pstride = PT[:, :, :, :].ap[0][0]
                                estride = E[:, :, :].ap[0][0]
                                nc.vector.tensor_mul(
                                    _ap(PT[:, :, :, :],
                                        [[pstride, 128], [512, 8],
                                         [384, 2], [1, 128]]),
                                    _ap(E[:, :, :],
                                        [[estride, 128], [512, 8],
                                         [256, 2], [1, 128]],
                                        doff=i * 128),
                                    _ap(mask_t[:, :],
                                        [[128, 128], [0, 8],
                                         [0, 2], [1, 128]]))
                                pvt = psD2.tile([81, 2048], F32,
                                                tag="pvt")
                                for pr in range(8):
                                    nc.tensor.matmul(
                                        pvt[:, pr * 256:(pr + 1) * 256],
                                        vt8[:, 2 * pr:2 * pr + 2,
                                            h * 84:h * 84 + 81],
                                        PT[:, pr, :, :],
                                        start=True, stop=True,
                                        perf_mode=DR)
                                attPt = pDw.tile([81, 2048], BF16,
                                                 tag="attPt", bufs=1)
                                nc.vector.tensor_copy(attPt, pvt)
                                dnm = pDw.tile([1, 2048], BF16, tag="dnmt",
                                               bufs=1)
                                nc.sync.dma_start(out=dnm,
                                                  in_=attPt[80:81, :])
                                dnr = pDw.tile([1, 2048], F32, tag="dnrt",
                                               bufs=1)
                                nc.vector.reciprocal(out=dnr, in_=dnm)
                                aB = pDw.tile([80, 2048], F32, tag="aBt",
                                              bufs=1)
                                nc.gpsimd.partition_broadcast(
                                    aB, dnr[0:1, :], channels=80)
                                aD8 = pDw.tile([80, 2048], FP8, tag="aDt8",
                                               bufs=1)
                                nc.vector.tensor_mul(aD8, attPt[0:80, :],
                                                     aB)
                                for (m, r, off, ln) in _head_rows(h):
                                    nc.sync.dma_start(
                                        out=attDt8[r:r + ln, m, :],
                                        in_=aD8[off:off + ln, :])
                    with tc.tile_pool(name="psD3", bufs=1,
                                      space="PSUM") as psD3:
                        for m, res in proj_dr(psD3, wot_t, attDt8, NQ):
                            (_, w, po), = res
                            oe = pDw.tile([128, NQ], F32, tag="oe", bufs=1)
                            nc.vector.scalar_tensor_tensor(
                                out=oe, in0=po, scalar=i256[:, 0:1],
                                in1=ht[:, m, :], op0=OP.mult, op1=OP.add)
                            nc.sync.dma_start(out=out_d[m, :, :], in_=oe)

            for it in range(reps):
                emit(it)

    nc.compile()
    _BUILD_CACHE[key] = nc
    return nc


def _prep_inputs(hidden_states, ln1_g, ln1_b, q1w, k1w, v1w, o1w, o1b,
                 ln3_g, ln3_b, ff_w1, ff_b1, ff_w2, ff_b2,
                 lnt_g, lnt_b, qtw, ktw, vtw, otw, otb):
    sc = 1.0 / math.sqrt(DH)

    def fold(g, b, w):
        return (np.asarray(g, np.float32)[:, None]
                * np.asarray(w, np.float32),
                np.asarray(b, np.float32) @ np.asarray(w, np.float32))

    wq, bq = fold(ln1_g, ln1_b, q1w)
    wq, bq = wq * sc, bq * sc
    wk, bk = fold(ln1_g, ln1_b, k1w)
    wv, bv = fold(ln1_g, ln1_b, v1w)
    wo1p = np.zeros((C, C), np.float32)
    idx = np.arange(DH)
    o1wf = np.asarray(o1w, np.float32)
    for h in range(H):
        wo1p[h * DH + idx, :] = o1wf[idx * H + h, :]
    wf1, bf1 = fold(ln3_g, ln3_b, ff_w1)
    bf1 = bf1 + np.asarray(ff_b1, np.float32)
    wqt, bqt = fold(lnt_g, lnt_b, qtw)
    wqt, bqt = wqt * sc, bqt * sc
    wkt, bkt = fold(lnt_g, lnt_b, ktw)
    wvt, bvt = fold(lnt_g, lnt_b, vtw)

    for name, b in (("o1b", o1b), ("ff_b2", ff_b2), ("otb", otb)):
        assert np.abs(np.asarray(b)).max() == 0, f"nonzero {name} unsupported"

    def w8(w, ncols=C):
        t = np.zeros((6, 128, ncols), np.float32)
        t[:KT] = (16 * w).reshape(KT, 128, ncols)
        return np.ascontiguousarray(t.transpose(1, 0, 2).astype(E4))

    def w8_20(w):
        return np.ascontiguousarray(
            (16 * w).reshape(NI, 128, C).transpose(1, 0, 2).astype(E4))

    mask = np.kron(np.eye(8, dtype=np.float32),
                   np.ones((F, F), np.float32))

    def f32(a):
        return np.ascontiguousarray(np.asarray(a, np.float32))

    shared = dict(
        wq=w8(wq), wk=w8(wk), wv=w8(wv), wo=w8(wo1p),
        wf1=w8(wf1, 2 * INNER), wf2=w8_20(np.asarray(ff_w2, np.float32)),
        wqt=w8(wqt), wkt=w8(wkt), wvt=w8(wvt),
        wot=w8(np.asarray(otw, np.float32)),
        bq=f32(bq), bk=f32(bk), bv16=f32(16 * bv),
        bf1g=f32(bf1[INNER:]), bf1a16=f32(16 * bf1[:INNER]),
        bqt=f32(bqt), bkt=f32(bkt), bvt16=f32(16 * bvt),
        mask=np.ascontiguousarray(mask.astype(ml_dtypes.bfloat16)),
    )

    hs = np.asarray(hidden_states, np.float32)
    bf = ml_dtypes.bfloat16
    in_maps = []
    for i in range(NCORES):
        fa, fb = 2 * i, 2 * i + 1
        fprev = max(2 * i - 1, 0)
        frames = [0, fprev, fa, fb]
        xum = hs[frames].reshape(NU, C).T
        m = dict(shared)
        m["xub"] = np.ascontiguousarray(xum.astype(bf).reshape(KT, 128, NU))
        in_maps.append(m)
    return in_maps


def kernel(**inputs):
    video_length = int(np.asarray(inputs.pop("video_length")))
    assert video_length == F, f"kernel hardcodes F={F}, got {video_length}"
    in_maps = _prep_inputs(**{k: np.asarray(v) for k, v in inputs.items()})
    nc = _build(reps=1)
    res = run_bass_kernel_spmd(nc, in_maps, list(range(NCORES)))
    out = np.empty((F, D, C), np.float32)
    for i in range(NCORES):
        r = res.results[i]["out"].reshape(C, D // NCORES, F)
        out[:, i * (D // NCORES):(i + 1) * (D // NCORES), :] = \
            r.transpose(2, 1, 0)
    return out
